# revision 15
# baseline (speedup 1.0000x reference)
"""Two-layer GCN (GCNConv -> relu -> GCNConv -> log_softmax) on 8 TRN2 NeuronCores.

Sharding: nodes (rows of x / aggregation outputs) are range-sharded across the
8 cores; edges are partitioned by destination core and sorted by destination so
scatter-add becomes a per-tile one-hot ("staircase") matmul into PSUM; source
node features are exchanged via AllGather of the (tiny) hidden tables.
"""

import os
import numpy as np

N = 100_000
E_RAW = 3_200_000
F_IN = 512
HID = 16
C = 10
N_CORES = 8
N_C = N // N_CORES          # nodes per core
P = 128
RJ = (N_C + P - 1) // P     # node tiles per core (98)
N_C_PAD = RJ * P            # padded nodes per core (12544)
V_PAD = N_CORES * N_C_PAD   # padded global table rows

_cache = {}


def _host_prep(edge_index, edge_weight):
    src = np.asarray(edge_index[0], dtype=np.int64)
    dst = np.asarray(edge_index[1], dtype=np.int64)
    w = np.asarray(edge_weight, dtype=np.float64)
    # GCNConv re-adds self-loops with weight 1
    loop = np.arange(N, dtype=np.int64)
    src = np.concatenate([src, loop])
    dst = np.concatenate([dst, loop])
    w = np.concatenate([w, np.ones(N, dtype=np.float64)])

    deg = np.bincount(dst, weights=w, minlength=N)
    dinv = np.where(deg > 0, 1.0 / np.sqrt(deg), 0.0)
    norm = (dinv[src] * w * dinv[dst]).astype(np.float32)

    core = dst // N_C
    per_core = []
    max_nt = 0
    max_span = 0
    max_refs = 2
    for c in range(N_CORES):
        m = core == c
        s_c = src[m]
        d_c = dst[m] - c * N_C
        n_c = norm[m]
        order = np.argsort(d_c, kind="stable")
        s_c, d_c, n_c = s_c[order], d_c[order], n_c[order]
        ec = len(s_c)
        nt = (ec + P - 1) // P
        max_nt = max(max_nt, nt)
        # dst span within each full 128-edge tile (padding handled later)
        nfull = ec // P
        if nfull:
            dv = d_c[: nfull * P].reshape(nfull, P)
            max_span = max(max_span, int((dv[:, -1] - dv[:, 0]).max()))
        if ec % P:
            tail = d_c[nfull * P:]
            max_span = max(max_span, int(tail[-1] - tail[0]))
        per_core.append((s_c, d_c, n_c))

    S = int(max(16, max_span + 1))
    grp = max(1, P // S)                            # tiles per PSUM group
    ch_tiles = grp * max(1, 64 // grp)              # tiles per gather chunk

    # uniform tile count across cores, pad for chunk granularity
    nt_pad = ((max_nt + ch_tiles - 1) // ch_tiles) * ch_tiles
    e_pad = nt_pad * P
    zero_row = nt_pad * S                           # slab row holding zeros

    cores_out = []
    for c in range(N_CORES):
        s_c, d_c, n_c = per_core[c]
        ec = len(s_c)
        # pad edges: src->0, norm->0, dst-> last dst (slot overwritten below)
        sp = np.zeros(e_pad, dtype=np.int64)
        dp = np.zeros(e_pad, dtype=np.int64)
        npd = np.zeros(e_pad, dtype=np.float32)
        sp[:ec] = s_c
        dp[:ec] = d_c
        dp[ec:] = d_c[-1] if ec else 0
        npd[:ec] = n_c
        base = dp[::P].copy()                       # [nt_pad] first dst of each tile
        slot = dp - np.repeat(base, P)              # [e_pad]
        cores_out.append((sp, dp, npd, slot, base, ec))

    inputs = []
    for c in range(N_CORES):
        sp, dp, npd, slot, base, ec = cores_out[c]
        # padding edges: any slot is safe (norm==0); clamp into [0,S)
        slot = np.clip(slot, 0, S - 1)
        # padded global table ids
        sp_pad = (sp // N_C) * N_C_PAD + (sp % N_C)
        # [P, nt] layouts: edge e = t*128 + p  ->  [p, t]
        srcidx = sp_pad.reshape(-1, P).T.astype(np.int32).copy()
        normv = npd.reshape(-1, P).T.astype(np.float32).copy()
        slotv = slot.reshape(-1, P).T.astype(np.float32).copy()

        # per-node assembly refs
        nodes = np.arange(N_C, dtype=np.int64)
        first_e = np.searchsorted(dp[:ec], nodes, side="left")
        last_e = np.searchsorted(dp[:ec], nodes, side="right") - 1
        has = last_e >= first_e
        t0 = first_e // P
        t1 = last_e // P
        if ec:
            assert (t1[has] - t0[has] <= max_refs - 1).all(), "node spans >2 tiles"
        refs = np.full((N_C_PAD, max_refs), zero_row, dtype=np.int64)
        r0 = t0 * S + (nodes - base[np.clip(t0, 0, nt_pad - 1)])
        r1 = t1 * S + (nodes - base[np.clip(t1, 0, nt_pad - 1)])
        ok0 = has & (nodes - base[np.clip(t0, 0, nt_pad - 1)] < S) & (nodes >= base[np.clip(t0, 0, nt_pad - 1)])
        assert ok0[has].all(), "slot out of range in first tile"
        refs[:N_C, 0] = np.where(has, r0, zero_row)
        two = has & (t1 > t0)
        refs[:N_C, 1] = np.where(two, r1, zero_row)
        # [P, RJ*max_refs]: node j*128+p, ref r -> col j*max_refs + r
        aref = refs.reshape(RJ, P, max_refs).transpose(1, 0, 2).reshape(P, RJ * max_refs)
        aref = np.ascontiguousarray(aref, dtype=np.int32)

        inputs.append(dict(srcidx=srcidx, normv=normv, slotv=slotv, aref=aref))

    meta = dict(S=S, NT=nt_pad, zero_row=zero_row, grp=grp, ch_tiles=ch_tiles)
    return inputs, meta


def _build_program(S, NT, GRP, CH_TILES):
    import concourse.bass as bass
    import concourse.bacc as bacc
    import concourse.tile as tile
    import concourse.mybir as mybir
    from concourse.masks import make_identity

    fp32 = mybir.dt.float32
    i32 = mybir.dt.int32
    AX = mybir.AxisListType.X
    AF = mybir.ActivationFunctionType
    OP = mybir.AluOpType

    NCH = NT // CH_TILES            # gather chunks per layer
    SLAB = NT * S + P               # slab rows (incl zero tail)
    KC = F_IN // P                  # contraction stripes (4)

    nc = bacc.Bacc("TRN2", target_bir_lowering=False, debug=False,
                   num_devices=N_CORES)

    xT = nc.dram_tensor("xT", [F_IN, N_C_PAD], fp32, kind="ExternalInput").ap()
    W1 = nc.dram_tensor("W1", [F_IN, HID], fp32, kind="ExternalInput").ap()
    b1b = nc.dram_tensor("b1b", [P, HID], fp32, kind="ExternalInput").ap()
    W2 = nc.dram_tensor("W2", [HID, C], fp32, kind="ExternalInput").ap()
    b2b = nc.dram_tensor("b2b", [P, C], fp32, kind="ExternalInput").ap()
    iota = nc.dram_tensor("iotaS", [P, S], fp32, kind="ExternalInput").ap()
    srcidx = nc.dram_tensor("srcidx", [P, NT], i32, kind="ExternalInput").ap()
    normv = nc.dram_tensor("normv", [P, NT], fp32, kind="ExternalInput").ap()
    slotv = nc.dram_tensor("slotv", [P, NT], fp32, kind="ExternalInput").ap()
    aref = nc.dram_tensor("aref", [P, RJ * 2], i32, kind="ExternalInput").ap()
    out = nc.dram_tensor("out", [N_C_PAD, C], fp32, kind="ExternalOutput").ap()

    with tile.TileContext(nc) as tc:
        with (
            tc.tile_pool(name="const", bufs=1) as cpool,
            tc.tile_pool(name="xs", bufs=2) as xpool,
            tc.tile_pool(name="upool", bufs=2) as upool,
            tc.tile_pool(name="gat", bufs=3) as gpool,
            tc.tile_pool(name="ed", bufs=3) as epool,
            tc.tile_pool(name="dr", bufs=4) as dpool,
            tc.tile_pool(name="asm", bufs=1) as apool,
            tc.tile_pool(name="psA", bufs=2, space="PSUM") as psA,
            tc.tile_pool(name="psE", bufs=2, space="PSUM") as psE,
            tc.tile_pool(name="psT", bufs=2, space="PSUM") as psT,
            tc.tile_pool(name="psD", bufs=1, space="PSUM") as psD,
            tc.tile_pool(name="dram", bufs=1, space="DRAM") as dram,
        ):
            # ---- constants resident in SBUF
            w1_s = cpool.tile([P, KC * HID], fp32, tag="w1")
            for kc in range(KC):
                nc.sync.dma_start(w1_s[:, kc * HID:(kc + 1) * HID],
                                  W1[kc * P:(kc + 1) * P, :])
            w2_s = cpool.tile([HID, C], fp32, tag="w2")
            nc.sync.dma_start(w2_s[:], W2[:])
            b1_s = cpool.tile([P, HID], fp32, tag="b1")
            nc.sync.dma_start(b1_s[:], b1b[:])
            b2_s = cpool.tile([P, C], fp32, tag="b2")
            nc.sync.dma_start(b2_s[:], b2b[:])
            iota_s = cpool.tile([P, S], fp32, tag="iota")
            nc.sync.dma_start(iota_s[:], iota[:])
            aref_s = cpool.tile([P, RJ * 2], i32, tag="aref")
            nc.sync.dma_start(aref_s[:], aref[:])
            ident = cpool.tile([P, P], fp32, tag="ident")
            make_identity(nc, ident[:])
            zt = cpool.tile([P, HID], fp32, tag="zt")
            nc.gpsimd.memset(zt[:], 0.0)

            # ---- internal DRAM
            u1_shard = dram.tile([N_C_PAD, HID], fp32)
            u1_full = dram.tile([V_PAD, HID], fp32)
            h1_shard = dram.tile([N_C_PAD, HID], fp32)
            h1_full = dram.tile([V_PAD, HID], fp32)
            slab1 = dram.tile([SLAB, HID], fp32)
            slab2 = dram.tile([SLAB, HID], fp32)

            # zero tails of slabs
            nc.sync.dma_start(slab1[NT * S: NT * S + P, :], zt[:])
            nc.sync.dma_start(slab2[NT * S: NT * S + P, :], zt[:])

            # ---- phase A: u1 = x @ W1 (shard rows)
            JG = 8
            JGP = JG * P
            for jg in range(RJ // JG + (1 if RJ % JG else 0)):
                j0 = jg * JG
                jn = min(JG, RJ - j0)
                xs = xpool.tile([P, KC * JGP], fp32, tag="xs")
                for kc in range(KC):
                    nc.sync.dma_start(
                        xs[:, kc * JGP: kc * JGP + jn * P],
                        xT[kc * P:(kc + 1) * P, j0 * P: j0 * P + jn * P])
                ug = upool.tile([P, JG * HID], fp32, tag="ug")
                for j in range(jn):
                    pu = psA.tile([P, HID], fp32, tag="pu")
                    for kc in range(KC):
                        nc.tensor.matmul(
                            out=pu[:],
                            lhsT=xs[:, kc * JGP + j * P: kc * JGP + (j + 1) * P],
                            rhs=w1_s[:, kc * HID:(kc + 1) * HID],
                            start=(kc == 0), stop=(kc == KC - 1))
                    nc.scalar.copy(ug[:, j * HID:(j + 1) * HID], pu[:])
                nc.sync.dma_start(
                    u1_shard[j0 * P: j0 * P + jn * P, :].rearrange(
                        "(j p) h -> p j h", p=P),
                    ug[:, : jn * HID].rearrange("p (j h) -> p j h", h=HID))

            # ---- all-gather u1
            nc.gpsimd.collective_compute(
                "AllGather", OP.bypass,
                replica_groups=[list(range(N_CORES))],
                ins=[u1_shard.opt()], outs=[u1_full.opt()])

            # ---- edge pass (shared for both layers)
            # PSB tiles accumulate into one [HID, PSB*S] psum batch (free-dim
            # windows), then drain via PE transpose to slot-major slab rows.
            PSB = 512 // S                  # tiles per psum batch (32 for S=16)
            TPC = P // S                    # tiles per transpose chunk (8)
            def edge_pass(table, slab, li):
                for ch in range(NCH):
                    t0 = ch * CH_TILES
                    idx_t = epool.tile([P, CH_TILES], i32, tag="eidx")
                    nrm_t = epool.tile([P, CH_TILES], fp32, tag="enrm")
                    slt_t = epool.tile([P, CH_TILES], fp32, tag="eslt")
                    nc.sync.dma_start(idx_t[:], srcidx[:, t0: t0 + CH_TILES])
                    nc.sync.dma_start(nrm_t[:], normv[:, t0: t0 + CH_TILES])
                    nc.sync.dma_start(slt_t[:], slotv[:, t0: t0 + CH_TILES])
                    g_t = gpool.tile([P, CH_TILES * HID], fp32, tag="gath")
                    for t in range(CH_TILES):
                        nc.gpsimd.indirect_dma_start(
                            out=g_t[:, t * HID:(t + 1) * HID],
                            out_offset=None, in_=table[:],
                            in_offset=bass.IndirectOffsetOnAxis(
                                ap=idx_t[:, t: t + 1], axis=0))
                    for pb in range(CH_TILES // PSB):
                        pb0 = pb * PSB      # tile offset of psum batch in chunk
                        psb = psE.tile([HID, PSB * S], fp32, tag="pe")
                        for g in range(PSB // GRP):
                            gt0 = pb0 + g * GRP
                            b8 = dpool.tile([P, GRP * S], fp32, tag="b8")
                            nc.vector.tensor_tensor(
                                out=b8[:].rearrange("p (g s) -> p g s", s=S),
                                in0=slt_t[:, gt0: gt0 + GRP].to_broadcast([P, GRP, S]),
                                in1=iota_s[:].unsqueeze(1).to_broadcast([P, GRP, S]),
                                op=OP.is_equal)
                            m8 = dpool.tile([P, GRP * HID], fp32, tag="m8")
                            nc.vector.tensor_tensor(
                                out=m8[:].rearrange("p (g h) -> p g h", h=HID),
                                in0=g_t[:, gt0 * HID:(gt0 + GRP) * HID].rearrange(
                                    "p (g h) -> p g h", h=HID),
                                in1=nrm_t[:, gt0: gt0 + GRP].to_broadcast([P, GRP, HID]),
                                op=OP.mult)
                            for i in range(GRP):
                                col = (g * GRP + i) * S
                                nc.tensor.matmul(
                                    out=psb[:, col: col + S],
                                    lhsT=m8[:, i * HID:(i + 1) * HID],
                                    rhs=b8[:, i * S:(i + 1) * S],
                                    start=True, stop=True)
                        dsb = dpool.tile([HID, PSB * S], fp32, tag="dsb")
                        nc.scalar.copy(dsb[:], psb[:])
                        for c in range(PSB * S // P):
                            pt2 = psT.tile([P, HID], fp32, tag="pt2")
                            nc.tensor.transpose(
                                out=pt2[:], in_=dsb[:, c * P:(c + 1) * P],
                                identity=ident[:HID, :HID])
                            dd = dpool.tile([P, HID], fp32, tag="dd")
                            nc.scalar.copy(dd[:], pt2[:])
                            row0 = (t0 + pb0) * S + c * P
                            nc.sync.dma_start(slab[row0: row0 + P, :], dd[:])

            # ---- assembly (gather per-node partials, combine)
            def assembly(slab, outbuf):
                a_t = apool.tile([P, RJ * 2 * HID], fp32, tag="asm")
                for col in range(RJ * 2):
                    nc.gpsimd.indirect_dma_start(
                        out=a_t[:, col * HID:(col + 1) * HID],
                        out_offset=None, in_=slab[:],
                        in_offset=bass.IndirectOffsetOnAxis(
                            ap=aref_s[:, col: col + 1], axis=0))
                v = a_t[:].rearrange("p (j r h) -> p j r h", r=2, h=HID)
                nc.vector.tensor_tensor(
                    out=outbuf[:].rearrange("p (j h) -> p j h", h=HID),
                    in0=v[:, :, 0, :], in1=v[:, :, 1, :], op=OP.add)

            edge_pass(u1_full, slab1, 0)
            h1 = apool.tile([P, RJ * HID], fp32, tag="h1")
            assembly(slab1, h1)
            # h1 = relu(agg + b1)
            nc.vector.tensor_tensor(
                out=h1[:].rearrange("p (j h) -> p j h", h=HID),
                in0=h1[:].rearrange("p (j h) -> p j h", h=HID),
                in1=b1_s[:].unsqueeze(1).to_broadcast([P, RJ, HID]),
                op=OP.add)
            nc.vector.tensor_scalar_max(h1[:], h1[:], 0.0)
            nc.sync.dma_start(
                h1_shard[:].rearrange("(j p) h -> p j h", p=P),
                h1[:].rearrange("p (j h) -> p j h", h=HID))

            nc.gpsimd.collective_compute(
                "AllGather", OP.bypass,
                replica_groups=[list(range(N_CORES))],
                ins=[h1_shard.opt()], outs=[h1_full.opt()])

            edge_pass(h1_full, slab2, 1)
            agg2 = apool.tile([P, RJ * HID], fp32, tag="agg2")
            assembly(slab2, agg2)

            # ---- out = log_softmax(agg2 @ W2 + b2)
            ob = apool.tile([P, RJ * C], fp32, tag="ob")
            for j in range(RJ):
                pt = psD.tile([HID, P], fp32, tag="ptr")
                nc.tensor.transpose(
                    out=pt[:], in_=agg2[:, j * HID:(j + 1) * HID], identity=ident[:])
                a2T = dpool.tile([HID, P], fp32, tag="a2T")
                nc.scalar.copy(a2T[:], pt[:])
                pz = psD.tile([P, C], fp32, tag="pz")
                nc.tensor.matmul(out=pz[:], lhsT=a2T[:], rhs=w2_s[:],
                                 start=True, stop=True)
                z = dpool.tile([P, C], fp32, tag="z")
                nc.vector.tensor_tensor(out=z[:], in0=pz[:], in1=b2_s[:], op=OP.add)
                mx = dpool.tile([P, 1], fp32, tag="mx")
                nc.vector.reduce_max(mx[:], z[:], axis=AX)
                nc.vector.tensor_tensor(out=z[:], in0=z[:],
                                        in1=mx[:].to_broadcast([P, C]),
                                        op=OP.subtract)
                ez = dpool.tile([P, C], fp32, tag="ez")
                nc.scalar.activation(ez[:], z[:], AF.Exp)
                sm = dpool.tile([P, 1], fp32, tag="sm")
                nc.vector.reduce_sum(sm[:], ez[:], axis=AX)
                lg = dpool.tile([P, 1], fp32, tag="lg")
                nc.scalar.activation(lg[:], sm[:], AF.Ln)
                nc.vector.tensor_tensor(out=ob[:, j * C:(j + 1) * C], in0=z[:],
                                        in1=lg[:].to_broadcast([P, C]),
                                        op=OP.subtract)
            nc.sync.dma_start(
                out[:].rearrange("(j p) c -> p j c", p=P),
                ob[:].rearrange("p (j c) -> p j c", c=C))

    nc.compile()
    return nc


def kernel(x, edge_index, edge_weight, W1, b1, W2, b2):
    from concourse.bass_utils import run_bass_kernel_spmd

    x = np.asarray(x, dtype=np.float32)
    W1 = np.asarray(W1, dtype=np.float32)
    b1 = np.asarray(b1, dtype=np.float32)
    W2 = np.asarray(W2, dtype=np.float32)
    b2 = np.asarray(b2, dtype=np.float32)

    per_core, meta = _host_prep(np.asarray(edge_index), np.asarray(edge_weight))
    S, NT = meta["S"], meta["NT"]

    key = (S, NT)
    if key not in _cache:
        _cache[key] = _build_program(S, NT, meta["grp"], meta["ch_tiles"])
    nc = _cache[key]

    iota_bc = np.tile(np.arange(S, dtype=np.float32)[None, :], (P, 1))
    b1b = np.tile(b1[None, :], (P, 1)).astype(np.float32)
    b2b = np.tile(b2[None, :], (P, 1)).astype(np.float32)

    in_maps = []
    for c in range(N_CORES):
        xc = np.zeros((F_IN, N_C_PAD), dtype=np.float32)
        xc[:, :N_C] = x[c * N_C:(c + 1) * N_C, :].T
        d = per_core[c]
        in_maps.append(dict(
            xT=np.ascontiguousarray(xc), W1=W1, b1b=b1b, W2=W2, b2b=b2b,
            iotaS=iota_bc, srcidx=d["srcidx"], normv=d["normv"],
            slotv=d["slotv"], aref=d["aref"]))

    trace = bool(int(os.environ.get("GNN_TRACE", "0")))
    res = run_bass_kernel_spmd(nc, in_maps, core_ids=list(range(N_CORES)),
                               trace=trace)
    global LAST_EXEC_TIME_NS
    LAST_EXEC_TIME_NS = res.exec_time_ns

    outs = [res.results[c]["out"][:N_C, :] for c in range(N_CORES)]
    return np.concatenate(outs, axis=0)


LAST_EXEC_TIME_NS = None
